# revision 1
# baseline (speedup 1.0000x reference)
"""Trainium2 Bass kernel for nn_AverageAttn (B=4, S=4096, D=H=1024, 8 cores).

out = igate * iQ + fgate * h, where
  avg  = causal cumulative average of iV along seq
  h    = relu(avg @ W1 + b1) @ W2 + b2
  ifg  = sigmoid(concat(iQ, h) @ Wg + bg);  igate, fgate = split(ifg)

Sharding: 8 cores = (batch b, seq half h).  Each core processes 2048 tokens.
Cores with h=1 also stream iV[b, :2048] to build the prefix chunk-sums.

On-device layout is "T-orientation": activations live as [feature, token]
tiles so matmuls chain without transposes; only iQ (in) and out (back) cross
orientation via PE transposes.  All matmul operands are float32r (TF32-like,
1 cycle/row at N>=256).
"""

import numpy as np

B, S, D = 4, 4096, 1024
H = 1024
T = S // 2              # tokens per core
P = 128
NCH = T // P            # 16 chunks of 128 tokens per core
NBLK = 4                # 512-token blocks per core
CPB = 4                 # chunks per block
ND = D // P             # 8 feature chunks
NG = 2 * D // P         # 16 gate chunks
NROW = 32               # S-table rows: 0..15 prefix, 16..31 shard chunks


def _host_constants():
    """Per-parity constants: scaled triangular blocks and carry masks."""
    consts = {}
    for half in (0, 1):
        off = half * T
        # ltri[t, c, s] = 1/(off + 128c + s + 1) if t <= s else 0
        ltri = np.zeros((P, NCH, P), np.float32)
        t = np.arange(P)[:, None]
        s = np.arange(P)[None, :]
        for c in range(NCH):
            denom = 1.0 / (off + P * c + s + 1).astype(np.float32)
            ltri[:, c, :] = np.where(t <= s, denom, 0.0)
        # mask[r, b, s] = 1/(off + 512b + s + 1) if S-row r feeds chunk of s
        mask = np.zeros((P, NBLK, 4 * P), np.float32)
        sb = np.arange(4 * P)
        for b in range(NBLK):
            w = 1.0 / (off + 4 * P * b + sb + 1).astype(np.float32)
            cc = sb // P  # chunk-in-block of each s
            for r in range(NROW):
                if r < 16:
                    inc = np.full(4 * P, half == 1)
                else:
                    inc = (r - 16) < (4 * b + cc)
                mask[r, b, :] = np.where(inc, w, 0.0)
        ltri_b = np.ascontiguousarray(
            ltri.reshape(P, NBLK, CPB, P).transpose(1, 0, 2, 3))
        mask_b = np.ascontiguousarray(mask.transpose(1, 0, 2))
        consts[half] = (ltri_b, mask_b)
    # oband[p, i] = 1 iff i == 32  ->  lhsT for S-row r is oband[:, 32-r:160-r]
    oband = np.zeros((P, 160), np.float32)
    oband[:, 32] = 1.0
    ident = np.eye(P, dtype=np.float32)
    return consts, oband, ident


def _build_program():
    import concourse.bass as bass  # noqa: F401
    import concourse.tile as tile
    from concourse import mybir, bacc

    f32 = mybir.dt.float32
    f32r = mybir.dt.float32r
    Relu = mybir.ActivationFunctionType.Relu
    Ident = mybir.ActivationFunctionType.Identity
    Sigm = mybir.ActivationFunctionType.Sigmoid

    nc = bacc.Bacc("TRN2", target_bir_lowering=False)

    q = nc.dram_tensor("q", [NCH, P, D], f32, kind="ExternalInput")
    v = nc.dram_tensor("v", [NCH, P, D], f32, kind="ExternalInput")
    vpre = nc.dram_tensor("vpre", [NCH, P, D], f32, kind="ExternalInput")
    w1s = nc.dram_tensor("w1s", [ND, P, ND, P], f32, kind="ExternalInput")
    w2s = nc.dram_tensor("w2s", [ND, P, ND, P], f32, kind="ExternalInput")
    wgs = nc.dram_tensor("wgs", [NG, P, NG, P], f32, kind="ExternalInput")
    b1c = nc.dram_tensor("b1c", [P, ND], f32, kind="ExternalInput")
    b2c = nc.dram_tensor("b2c", [P, ND], f32, kind="ExternalInput")
    bgc = nc.dram_tensor("bgc", [P, NG], f32, kind="ExternalInput")
    ltri = nc.dram_tensor("ltri", [NBLK, P, CPB, P], f32, kind="ExternalInput")
    maskd = nc.dram_tensor("maskd", [NBLK, P, 4 * P], f32, kind="ExternalInput")
    oband = nc.dram_tensor("oband", [P, 160], f32, kind="ExternalInput")
    ident = nc.dram_tensor("ident", [P, P], f32, kind="ExternalInput")
    o = nc.dram_tensor("o", [NCH, P, D], f32, kind="ExternalOutput")

    with tile.TileContext(nc) as tc:
        import contextlib
        ctx = contextlib.ExitStack()
        with ctx:
            cpool = ctx.enter_context(tc.tile_pool(name="consts", bufs=1))
            vpool = ctx.enter_context(tc.tile_pool(name="vq", bufs=4))
            qpool = ctx.enter_context(tc.tile_pool(name="qp", bufs=2))
            wpool = ctx.enter_context(tc.tile_pool(name="wslab", bufs=2))
            apool = ctx.enter_context(tc.tile_pool(name="acts", bufs=2))
            a1pool = ctx.enter_context(tc.tile_pool(name="acts1", bufs=1))
            avpool = ctx.enter_context(tc.tile_pool(name="avsl", bufs=16))
            spool = ctx.enter_context(tc.tile_pool(name="small", bufs=2))
            opool = ctx.enter_context(tc.tile_pool(name="outs", bufs=1))
            ps_mm = ctx.enter_context(tc.tile_pool(name="psmm", bufs=2, space="PSUM"))
            ps_cum = ctx.enter_context(tc.tile_pool(name="pscum", bufs=2, space="PSUM"))
            ps_tr = ctx.enter_context(tc.tile_pool(name="pstr", bufs=2, space="PSUM"))
            ps_sp = ctx.enter_context(tc.tile_pool(name="pssp", bufs=1, space="PSUM"))

            # ---- constants -------------------------------------------------
            identT = cpool.tile([P, P], f32r, tag="ident")
            nc.sync.dma_start(identT[:], ident[:].bitcast(f32r))
            obandT = cpool.tile([P, 160], f32r, tag="oband")
            nc.sync.dma_start(obandT[:], oband[:].bitcast(f32r))
            b1T = cpool.tile([P, ND], f32, tag="b1")
            nc.sync.dma_start(b1T[:], b1c[:])
            b2T = cpool.tile([P, ND], f32, tag="b2")
            nc.sync.dma_start(b2T[:], b2c[:])
            bgT = cpool.tile([P, NG], f32, tag="bg")
            nc.sync.dma_start(bgT[:], bgc[:])

            S_sb = cpool.tile([P, D], f32r, tag="Ssb")

            def srow_lhsT(r):
                return obandT[:, 32 - r:160 - r]

            # ---- prefix pass: S rows 0..15 from vpre ----------------------
            sp = ps_sp.tile([P, D], f32, tag="sp")
            for c in range(NCH):
                vch = vpool.tile([P, D], f32r, tag="vch")
                nc.sync.dma_start(vch[:], vpre[c].bitcast(f32r))
                for hf in range(2):
                    nc.tensor.matmul(
                        sp[:, hf * 512:(hf + 1) * 512],
                        srow_lhsT(c),
                        vch[:, hf * 512:(hf + 1) * 512],
                        start=(c == 0), stop=(c == NCH - 1),
                        skip_group_check=True,
                    )
            nc.vector.tensor_copy(S_sb[:], sp[:])

            # ---- main: 2 pairs of 512-token blocks -------------------------
            iqT = {}
            hT = {}
            ig_sb = {}
            outT = {}
            avgT = {}
            h1T = {}

            streams = {}

            def stream_block(blk):
                vchs, qchs = [], []
                for cc in range(CPB):
                    c = blk * CPB + cc
                    vch = vpool.tile([P, D], f32r, tag="vch")
                    nc.sync.dma_start(vch[:], v[c].bitcast(f32r))
                    vchs.append(vch)
                    qch = qpool.tile([P, D], f32r, tag="qch")
                    nc.sync.dma_start(qch[:], q[c].bitcast(f32r))
                    qchs.append(qch)
                streams[blk] = (vchs, qchs)

            def scan_block(blk):
                """S-rows, iQ transposes, cumulative average."""
                vchs, qchs = streams.pop(blk)
                ltb = wpool.tile([P, CPB, P], f32r, tag="ltri")
                nc.sync.dma_start(ltb[:], ltri[blk].bitcast(f32r))
                mkb = wpool.tile([P, 4 * P], f32r, tag="mask")
                nc.sync.dma_start(mkb[:], maskd[blk].bitcast(f32r))

                sp = ps_sp.tile([P, D], f32, tag="sp")
                for cc in range(CPB):
                    r = 16 + blk * CPB + cc
                    for hf in range(2):
                        nc.tensor.matmul(
                            sp[:, hf * 512:(hf + 1) * 512],
                            srow_lhsT(r),
                            vchs[cc][:, hf * 512:(hf + 1) * 512],
                            start=(cc == 0), stop=(cc == CPB - 1),
                            skip_group_check=True,
                        )
                nc.vector.tensor_add(S_sb[:], S_sb[:], sp[:])

                iqT[blk] = apool.tile([P, ND, 4 * P], f32r, tag="iqT", name="iqT")
                for tc in range(CPB):
                    for dh in range(2):
                        ptr = ps_tr.tile([P, 4 * P], f32r, tag="tr")
                        for dd in range(4):
                            d = dh * 4 + dd
                            nc.tensor.transpose(
                                ptr[:, dd * P:(dd + 1) * P],
                                qchs[tc][:, d * P:(d + 1) * P],
                                identT[:],
                            )
                        nc.vector.tensor_copy(
                            iqT[blk][:, dh * 4:(dh + 1) * 4, tc * P:(tc + 1) * P],
                            ptr[:].rearrange("p (a b) -> p a b", a=4))

                avgT[blk] = []
                for d in range(ND):
                    pav = ps_cum.tile([P, 4 * P], f32, tag="avg")
                    # cc=0 clears the whole bank (start=True); cc=1..3 land on
                    # has_written=0 slices (overwrite); carry accumulates last.
                    for cc in range(CPB):
                        nc.tensor.matmul(
                            pav[:, cc * P:(cc + 1) * P],
                            vchs[cc][:, d * P:(d + 1) * P],
                            ltb[:, cc, :],
                            start=(cc == 0), stop=False,
                            skip_group_check=True,
                        )
                    nc.tensor.matmul(
                        pav[:],
                        S_sb[:, d * P:(d + 1) * P],
                        mkb[:],
                        start=False, stop=True,
                        skip_group_check=True,
                    )
                    avsl = avpool.tile([P, 4 * P], f32r, tag="avgT",
                                       name="avsl")
                    nc.scalar.copy(avsl[:], pav[:])
                    avgT[blk].append(avsl)

            def ffn1_pair(blocks):
                for blk in blocks:
                    h1T[blk] = apool.tile([P, ND, 4 * P], f32r, tag="h1T",
                                          name="h1T")
                for j in range(ND):
                    w1t = wpool.tile([P, ND, P], f32r, tag="w12")
                    nc.sync.dma_start(w1t[:], w1s[j].bitcast(f32r))
                    for blk in blocks:
                        pm = ps_mm.tile([P, 4 * P], f32, tag="mm")
                        for d in range(ND):
                            nc.tensor.matmul(
                                pm[:], w1t[:, d, :], avgT[blk][d][:],
                                start=(d == 0), stop=(d == ND - 1),
                            )
                        nc.scalar.activation(h1T[blk][:, j, :], pm[:], Relu,
                                             bias=b1T[:, j:j + 1])

            def ffn2_pair(blocks):
                for blk in blocks:
                    hT[blk] = apool.tile([P, ND, 4 * P], f32r, tag="hT",
                                         name="hT")
                for d2 in range(ND):
                    w2t = wpool.tile([P, ND, P], f32r, tag="w12")
                    nc.sync.dma_start(w2t[:], w2s[d2].bitcast(f32r))
                    for blk in blocks:
                        pm = ps_mm.tile([P, 4 * P], f32, tag="mm")
                        for j in range(ND):
                            nc.tensor.matmul(
                                pm[:], w2t[:, j, :], h1T[blk][:, j, :],
                                start=(j == 0), stop=(j == ND - 1),
                            )
                        nc.scalar.activation(hT[blk][:, d2, :], pm[:], Ident,
                                             bias=b2T[:, d2:d2 + 1])

            for pair in range(2):
                blkA, blkB = 2 * pair, 2 * pair + 1
                blocks = (blkA, blkB)
                stream_block(blkA)
                scan_block(blkA)
                stream_block(blkB)
                scan_block(blkB)
                ffn1_pair(blocks)
                ffn2_pair(blocks)

                for blk in blocks:
                    outT[blk] = apool.tile([P, ND, 4 * P], f32r, tag="h1T",
                                           name="outT")
                for gp in range(ND):
                    for gg in (gp, gp + ND):
                        wgt = wpool.tile([P, NG, P], f32r, tag="wg")
                        nc.sync.dma_start(wgt[:], wgs[gg].bitcast(f32r))
                        for blk in blocks:
                            pg = ps_mm.tile([P, 4 * P], f32, tag="mm")
                            for c in range(NG):
                                rhs = (iqT[blk][:, c, :] if c < ND
                                       else hT[blk][:, c - ND, :])
                                nc.tensor.matmul(
                                    pg[:], wgt[:, c, :], rhs,
                                    start=(c == 0), stop=(c == NG - 1),
                                )
                            gate = spool.tile([P, 4 * P], f32r,
                                              tag=("ig" if gg < ND else "fg"))
                            nc.scalar.activation(gate[:], pg[:], Sigm,
                                                 bias=bgT[:, gg:gg + 1])
                            if gg < ND:
                                ig_sb[blk] = gate
                            else:
                                # final elementwise + transpose-out this d-chunk
                                ot = outT[blk][:, gp, :]
                                tmp = spool.tile([P, 4 * P], f32r, tag="tmp")
                                nc.vector.tensor_mul(
                                    tmp[:], ig_sb[blk][:], iqT[blk][:, gp, :])
                                nc.vector.tensor_mul(
                                    ot, gate[:], hT[blk][:, gp, :])
                                nc.vector.tensor_add(ot, ot, tmp[:])
                                ptr = ps_tr.tile([P, 4 * P], f32r, tag="tr")
                                for tc in range(CPB):
                                    nc.tensor.transpose(
                                        ptr[:, tc * P:(tc + 1) * P],
                                        outT[blk][:, gp, tc * P:(tc + 1) * P],
                                        identT[:],
                                    )
                                otp = spool.tile([P, CPB, P], f32, tag="otp")
                                nc.vector.tensor_copy(
                                    otp[:], ptr[:].rearrange("p (a b) -> p a b", a=CPB))
                                nc.sync.dma_start(
                                    o[blk * CPB:(blk + 1) * CPB, :,
                                      gp * P:(gp + 1) * P].rearrange("c p d -> p c d"),
                                    otp[:])

    nc.finalize()
    return nc


_CACHED = {}
_last_result = None


def kernel(iQ, iV, W1, b1, W2, b2, Wg, bg):
    import sys
    if '/opt/trn_rl_repo' not in sys.path:
        sys.path.insert(0, '/opt/trn_rl_repo')
    from concourse.bass_utils import run_bass_kernel_spmd

    iQ = np.asarray(iQ, np.float32)
    iV = np.asarray(iV, np.float32)
    W1 = np.asarray(W1, np.float32)
    b1 = np.asarray(b1, np.float32)
    W2 = np.asarray(W2, np.float32)
    b2 = np.asarray(b2, np.float32)
    Wg = np.asarray(Wg, np.float32)
    bg = np.asarray(bg, np.float32)

    if 'nc' not in _CACHED:
        _CACHED['nc'] = _build_program()
    nc = _CACHED['nc']

    consts, oband, ident = _host_constants()

    # weight slabs: lhsT tiles, slab[m][p, k, q] = W[k*128+p, m*128+q]
    def slabs(W, n):
        return np.ascontiguousarray(
            W.reshape(n, P, n, P).transpose(2, 1, 0, 3))

    w1s = slabs(W1, ND)
    w2s = slabs(W2, ND)
    wgs = slabs(Wg, NG)
    b1c = np.ascontiguousarray(b1.reshape(ND, P).T)
    b2c = np.ascontiguousarray(b2.reshape(ND, P).T)
    bgc = np.ascontiguousarray(bg.reshape(NG, P).T)
    zpre = np.zeros((NCH, P, D), np.float32)

    in_maps = []
    for core in range(8):
        b, half = core // 2, core % 2
        ltri_h, mask_h = consts[half]
        in_maps.append({
            "q": np.ascontiguousarray(
                iQ[b, half * T:(half + 1) * T].reshape(NCH, P, D)),
            "v": np.ascontiguousarray(
                iV[b, half * T:(half + 1) * T].reshape(NCH, P, D)),
            "vpre": (np.ascontiguousarray(iV[b, :T].reshape(NCH, P, D))
                     if half == 1 else zpre),
            "w1s": w1s, "w2s": w2s, "wgs": wgs,
            "b1c": b1c, "b2c": b2c, "bgc": bgc,
            "ltri": ltri_h, "maskd": mask_h,
            "oband": oband, "ident": ident,
        })

    res = run_bass_kernel_spmd(nc, in_maps, core_ids=list(range(8)))
    global _last_result
    _last_result = res

    out = np.empty((B, S, D), np.float32)
    for core in range(8):
        b, half = core // 2, core % 2
        out[b, half * T:(half + 1) * T] = res.results[core]["o"].reshape(T, D)
    return out



# revision 2
# speedup vs baseline: 2.0176x; 2.0176x over previous
"""Trainium2 Bass kernel for nn_AverageAttn (B=4, S=4096, D=H=1024, 8 cores).

out = igate * iQ + fgate * h, where
  avg  = causal cumulative average of iV along seq
  h    = relu(avg @ W1 + b1) @ W2 + b2
  ifg  = sigmoid(concat(iQ, h) @ Wg + bg);  igate, fgate = split(ifg)

Sharding: 8 cores = (batch b, seq half hf).  Each core processes 2048 tokens.
Cores with hf=1 also stream iV[b, :2048] to build the prefix chunk-sums.

Dtype strategy (tolerance is 2e-2):
  - FFN1/FFN2/gate matmuls run in fp8e4 with DoubleRow perf mode
    (2 contraction rows per PE cycle, 2x over bf16/f32r).
  - cumulative-average matmuls (ltri / carry) run in bf16 (1 cycle/row at
    any output width; f32r would pay 4x on the 128-wide outputs).
  - iQ is pre-transposed on host and uploaded both as bf16 (final
    elementwise) and fp8 (gate matmul rhs) - no PE transposes needed.
  - output is written feature-major in bf16 and re-laid-out on host.
All matmul accumulation stays f32 in PSUM.
"""

import numpy as np

B, S, D = 4, 4096, 1024
H = 1024
T = S // 2              # tokens per core
P = 128
NCH = T // P            # 16 chunks of 128 tokens per core
NBLK = 4                # 512-token blocks per core
CPB = 4                 # chunks per block
TB = CPB * P            # 512 tokens per block
ND = D // P             # 8 feature chunks
NG = 2 * D // P         # 16 gate chunks
NROW = 32               # S-table rows: 0..15 prefix, 16..31 shard chunks


def _host_constants():
    """Per-parity constants: scaled triangular blocks and carry masks."""
    import ml_dtypes
    bf16 = ml_dtypes.bfloat16
    consts = {}
    for half in (0, 1):
        off = half * T
        # ltri[t, c, s] = 1/(off + 128c + s + 1) if t <= s else 0
        ltri = np.zeros((P, NCH, P), np.float32)
        t = np.arange(P)[:, None]
        s = np.arange(P)[None, :]
        for c in range(NCH):
            denom = 1.0 / (off + P * c + s + 1).astype(np.float32)
            ltri[:, c, :] = np.where(t <= s, denom, 0.0)
        # mask[r, b, s] = 1/(off + 512b + s + 1) if S-row r feeds chunk of s
        mask = np.zeros((P, NBLK, TB), np.float32)
        sb = np.arange(TB)
        for b in range(NBLK):
            w = 1.0 / (off + TB * b + sb + 1).astype(np.float32)
            cc = sb // P  # chunk-in-block of each s
            for r in range(NROW):
                if r < 16:
                    inc = np.full(TB, half == 1)
                else:
                    inc = (r - 16) < (4 * b + cc)
                mask[r, b, :] = np.where(inc, w, 0.0)
        ltri_b = np.ascontiguousarray(
            ltri.reshape(P, NBLK, CPB, P).transpose(1, 0, 2, 3)).astype(bf16)
        mask_b = np.ascontiguousarray(mask.transpose(1, 0, 2)).astype(bf16)
        consts[half] = (ltri_b, mask_b)
    # oband[p, i] = 1 iff i == 32  ->  lhsT for S-row r is oband[:, 32-r:160-r]
    oband = np.zeros((P, 160), bf16)
    oband[:, 32] = 1.0
    return consts, oband


def _build_program():
    import concourse.bass as bass  # noqa: F401
    import concourse.tile as tile
    from concourse import mybir, bacc

    f32 = mybir.dt.float32
    bf16 = mybir.dt.bfloat16
    f8 = mybir.dt.float8e4
    Relu = mybir.ActivationFunctionType.Relu
    Ident = mybir.ActivationFunctionType.Identity
    Sigm = mybir.ActivationFunctionType.Sigmoid
    DR = mybir.MatmulPerfMode.DoubleRow

    nc = bacc.Bacc("TRN2", target_bir_lowering=False)

    v = nc.dram_tensor("v", [NCH, P, D], bf16, kind="ExternalInput")
    vpre = nc.dram_tensor("vpre", [NCH, P, D], bf16, kind="ExternalInput")
    qt8 = nc.dram_tensor("qt8", [P, ND, T], f8, kind="ExternalInput")
    qtb = nc.dram_tensor("qtb", [P, ND, T], bf16, kind="ExternalInput")
    w1s = nc.dram_tensor("w1s", [ND, P, ND, P], f8, kind="ExternalInput")
    w2s = nc.dram_tensor("w2s", [ND, P, ND, P], f8, kind="ExternalInput")
    wgs = nc.dram_tensor("wgs", [NG, P, NG, P], f8, kind="ExternalInput")
    b1c = nc.dram_tensor("b1c", [P, ND], f32, kind="ExternalInput")
    b2c = nc.dram_tensor("b2c", [P, ND], f32, kind="ExternalInput")
    bgc = nc.dram_tensor("bgc", [P, NG], f32, kind="ExternalInput")
    ltri = nc.dram_tensor("ltri", [NBLK, P, CPB, P], bf16, kind="ExternalInput")
    maskd = nc.dram_tensor("maskd", [NBLK, P, TB], bf16, kind="ExternalInput")
    oband = nc.dram_tensor("oband", [P, 160], bf16, kind="ExternalInput")
    o = nc.dram_tensor("o", [ND, P, T], bf16, kind="ExternalOutput")

    with tile.TileContext(nc) as tc:
        import contextlib
        ctx = contextlib.ExitStack()
        with ctx:
            cpool = ctx.enter_context(tc.tile_pool(name="consts", bufs=1))
            vpool = ctx.enter_context(tc.tile_pool(name="vq", bufs=6))
            qpool = ctx.enter_context(tc.tile_pool(name="qp", bufs=2))
            mpool = ctx.enter_context(tc.tile_pool(name="masks", bufs=2))
            apool = ctx.enter_context(tc.tile_pool(name="acts", bufs=2))
            spool = ctx.enter_context(tc.tile_pool(name="small", bufs=3))
            ps_sp = ctx.enter_context(tc.tile_pool(name="pssp", bufs=1, space="PSUM"))
            ps_cum = ctx.enter_context(tc.tile_pool(name="pscum", bufs=2, space="PSUM"))
            ps_mm = ctx.enter_context(tc.tile_pool(name="psmm", bufs=4, space="PSUM"))

            # ---- constants + resident weights ------------------------------
            obandT = cpool.tile([P, 160], bf16, tag="oband")
            nc.sync.dma_start(obandT[:], oband[:])
            b1T = cpool.tile([P, ND], f32, tag="b1")
            nc.sync.dma_start(b1T[:], b1c[:])
            b2T = cpool.tile([P, ND], f32, tag="b2")
            nc.sync.dma_start(b2T[:], b2c[:])
            bgT = cpool.tile([P, NG], f32, tag="bg")
            nc.sync.dma_start(bgT[:], bgc[:])

            w1t = cpool.tile([P, ND, ND, P], f8, tag="w1")
            w2t = cpool.tile([P, ND, ND, P], f8, tag="w2")
            wgt = cpool.tile([P, NG, NG, P], f8, tag="wg")
            for j in range(ND):
                nc.sync.dma_start(w1t[:, j], w1s[j])
                nc.sync.dma_start(w2t[:, j], w2s[j])
            for g in range(NG):
                nc.sync.dma_start(wgt[:, g], wgs[g])

            S_sb = cpool.tile([P, D], f32, tag="Ssb")
            S8b = cpool.tile([P, D], bf16, tag="S8b")

            def srow_lhsT(r):
                return obandT[:, 32 - r:160 - r]

            # ---- prefix pass: S rows 0..15 from vpre ----------------------
            sp = ps_sp.tile([P, D], f32, tag="sp")
            for c in range(NCH):
                vch = vpool.tile([P, D], bf16, tag="vch")
                nc.sync.dma_start(vch[:], vpre[c])
                for hf in range(2):
                    nc.tensor.matmul(
                        sp[:, hf * 512:(hf + 1) * 512],
                        srow_lhsT(c),
                        vch[:, hf * 512:(hf + 1) * 512],
                        start=(c == 0), stop=(c == NCH - 1),
                        skip_group_check=True,
                    )
            nc.vector.tensor_copy(S_sb[:], sp[:])

            # ---- main: 4 blocks of 512 tokens ------------------------------
            for blk in range(NBLK):
                vchs = []
                for cc in range(CPB):
                    vch = vpool.tile([P, D], bf16, tag="vch")
                    nc.sync.dma_start(vch[:], v[blk * CPB + cc])
                    vchs.append(vch)
                ltb = mpool.tile([P, CPB, P], bf16, tag="ltri")
                nc.sync.dma_start(ltb[:], ltri[blk])
                mkb = mpool.tile([P, TB], bf16, tag="mask")
                nc.sync.dma_start(mkb[:], maskd[blk])
                qbt = qpool.tile([P, ND, TB], bf16, tag="qbt")
                nc.sync.dma_start(qbt[:], qtb[:, :, blk * TB:(blk + 1) * TB])
                q8t = qpool.tile([P, ND, TB], f8, tag="q8t")
                nc.sync.dma_start(q8t[:], qt8[:, :, blk * TB:(blk + 1) * TB])

                # S rows for this block's 4 chunks, then fold into S_sb
                sp = ps_sp.tile([P, D], f32, tag="sp")
                for cc in range(CPB):
                    r = 16 + blk * CPB + cc
                    for hf in range(2):
                        nc.tensor.matmul(
                            sp[:, hf * 512:(hf + 1) * 512],
                            srow_lhsT(r),
                            vchs[cc][:, hf * 512:(hf + 1) * 512],
                            start=(cc == 0), stop=(cc == CPB - 1),
                            skip_group_check=True,
                        )
                nc.vector.tensor_add(S_sb[:], S_sb[:], sp[:])
                nc.vector.tensor_copy(S8b[:], S_sb[:])

                # cumulative average -> fp8 [feature, token] tiles
                avg8 = apool.tile([P, ND, TB], f8, tag="avg8", name="avg8")
                for d in range(ND):
                    pav = ps_cum.tile([P, TB], f32, tag="avg")
                    # cc=0 clears the whole bank (start=True); cc=1..3 land on
                    # has_written=0 slices (overwrite); carry accumulates last.
                    for cc in range(CPB):
                        nc.tensor.matmul(
                            pav[:, cc * P:(cc + 1) * P],
                            vchs[cc][:, d * P:(d + 1) * P],
                            ltb[:, cc, :],
                            start=(cc == 0), stop=False,
                            skip_group_check=True,
                        )
                    nc.tensor.matmul(
                        pav[:],
                        S8b[:, d * P:(d + 1) * P],
                        mkb[:],
                        start=False, stop=True,
                        skip_group_check=True,
                    )
                    nc.scalar.copy(avg8[:, d, :], pav[:])

                # FFN1: h1 = relu(avg @ W1 + b1)   (fp8 DoubleRow)
                h18 = apool.tile([P, ND, TB], f8, tag="h18", name="h18")
                for j in range(ND):
                    pm = ps_mm.tile([P, TB], f32, tag="mm")
                    for i in range(4):
                        nc.tensor.matmul(
                            pm[:], w1t[:, j, 2 * i:2 * i + 2, :],
                            avg8[:, 2 * i:2 * i + 2, :],
                            start=(i == 0), stop=(i == 3), perf_mode=DR,
                        )
                    nc.scalar.activation(h18[:, j, :], pm[:], Relu,
                                         bias=b1T[:, j:j + 1])

                # FFN2: h = h1 @ W2 + b2  (bf16 copy for elementwise, fp8 for gate)
                hTb = apool.tile([P, ND, TB], bf16, tag="hTb", name="hTb")
                h8 = apool.tile([P, ND, TB], f8, tag="h8", name="h8")
                for d2 in range(ND):
                    pm = ps_mm.tile([P, TB], f32, tag="mm")
                    for i in range(4):
                        nc.tensor.matmul(
                            pm[:], w2t[:, d2, 2 * i:2 * i + 2, :],
                            h18[:, 2 * i:2 * i + 2, :],
                            start=(i == 0), stop=(i == 3), perf_mode=DR,
                        )
                    nc.scalar.activation(hTb[:, d2, :], pm[:], Ident,
                                         bias=b2T[:, d2:d2 + 1])
                    nc.vector.tensor_copy(h8[:, d2, :], hTb[:, d2, :])

                # gate + final elementwise, one feature chunk at a time
                ig_sb = None
                for gp in range(ND):
                    for gg in (gp, gp + ND):
                        pg = ps_mm.tile([P, TB], f32, tag="mm")
                        for i in range(4):
                            nc.tensor.matmul(
                                pg[:], wgt[:, gg, 2 * i:2 * i + 2, :],
                                q8t[:, 2 * i:2 * i + 2, :],
                                start=(i == 0), stop=False, perf_mode=DR,
                            )
                        for i in range(4):
                            nc.tensor.matmul(
                                pg[:], wgt[:, gg, ND + 2 * i:ND + 2 * i + 2, :],
                                h8[:, 2 * i:2 * i + 2, :],
                                start=False, stop=(i == 3), perf_mode=DR,
                            )
                        gate = spool.tile([P, TB], bf16,
                                          tag=("ig" if gg < ND else "fg"))
                        nc.scalar.activation(gate[:], pg[:], Sigm,
                                             bias=bgT[:, gg:gg + 1])
                        if gg < ND:
                            ig_sb = gate
                        else:
                            tmp = spool.tile([P, TB], bf16, tag="tmp")
                            nc.vector.tensor_mul(tmp[:], ig_sb[:],
                                                 qbt[:, gp, :])
                            ot = spool.tile([P, TB], bf16, tag="ot")
                            nc.vector.tensor_mul(ot[:], gate[:], hTb[:, gp, :])
                            nc.vector.tensor_add(ot[:], ot[:], tmp[:])
                            nc.sync.dma_start(
                                o[gp, :, blk * TB:(blk + 1) * TB], ot[:])

    nc.finalize()
    return nc


_CACHED = {}
_last_result = None


def kernel(iQ, iV, W1, b1, W2, b2, Wg, bg):
    import sys
    if '/opt/trn_rl_repo' not in sys.path:
        sys.path.insert(0, '/opt/trn_rl_repo')
    from concourse.bass_utils import run_bass_kernel_spmd
    import ml_dtypes

    bf16 = ml_dtypes.bfloat16
    f8 = ml_dtypes.float8_e4m3

    iQ = np.asarray(iQ, np.float32)
    iV = np.asarray(iV, np.float32)
    W1 = np.asarray(W1, np.float32)
    b1 = np.asarray(b1, np.float32)
    W2 = np.asarray(W2, np.float32)
    b2 = np.asarray(b2, np.float32)
    Wg = np.asarray(Wg, np.float32)
    bg = np.asarray(bg, np.float32)

    if 'nc' not in _CACHED:
        _CACHED['nc'] = _build_program()
    nc = _CACHED['nc']

    consts, oband = _host_constants()

    # weight slabs: lhsT tiles, slab[m][p, k, q] = W[k*128+p, m*128+q]
    def slabs(W, n):
        return np.ascontiguousarray(
            W.reshape(n, P, n, P).transpose(2, 1, 0, 3)).astype(f8)

    w1s = slabs(W1, ND)
    w2s = slabs(W2, ND)
    wgs = slabs(Wg, NG)
    b1c = np.ascontiguousarray(b1.reshape(ND, P).T)
    b2c = np.ascontiguousarray(b2.reshape(ND, P).T)
    bgc = np.ascontiguousarray(bg.reshape(NG, P).T)
    zpre = np.zeros((NCH, P, D), bf16)

    in_maps = []
    for core in range(8):
        b, half = core // 2, core % 2
        ltri_h, mask_h = consts[half]
        iQs = iQ[b, half * T:(half + 1) * T]          # [T, D]
        qtb = np.ascontiguousarray(
            iQs.T.reshape(ND, P, T).transpose(1, 0, 2)).astype(bf16)
        qt8 = qtb.astype(f8)
        in_maps.append({
            "qtb": qtb, "qt8": qt8,
            "v": np.ascontiguousarray(
                iV[b, half * T:(half + 1) * T].reshape(NCH, P, D)).astype(bf16),
            "vpre": (np.ascontiguousarray(
                iV[b, :T].reshape(NCH, P, D)).astype(bf16)
                if half == 1 else zpre),
            "w1s": w1s, "w2s": w2s, "wgs": wgs,
            "b1c": b1c, "b2c": b2c, "bgc": bgc,
            "ltri": ltri_h, "maskd": mask_h,
            "oband": oband,
        })

    res = run_bass_kernel_spmd(nc, in_maps, core_ids=list(range(8)))
    global _last_result
    _last_result = res

    out = np.empty((B, S, D), np.float32)
    for core in range(8):
        b, half = core // 2, core % 2
        ot = res.results[core]["o"].astype(np.float32)   # [ND, P, T]
        out[b, half * T:(half + 1) * T] = (
            ot.transpose(2, 0, 1).reshape(T, D))
    return out
